# revision 42
# baseline (speedup 1.0000x reference)
"""Trainium2 Bass kernel for nn_MixModule (moe_routing).

Math: the reference computes outs[b,s,o,f] = sum_d x[b,s,d]*W[o,f,d] + b[o,f],
then y = sum_o weights[o]*outs[...,o,:].  This collapses to a single affine map

    W_eff[f,d] = sum_o weights[o] * W[o,f,d]
    b_eff[f]   = sum_o weights[o] * b[o,f]
    y          = x @ W_eff.T + b_eff

Sharding: data-parallel over tokens, 16384 tokens per core across 8 cores;
W/b replicated; no cross-core communication.

fp16 end-to-end on the wire (rel-err budget is 2e-2; fp16 quantization of x
and y costs ~4e-4), which halves HBM traffic vs fp32: 4 MiB in + 4 MiB out
per core ~= 19.7 us at the ~427 GB/s/core DMA ceiling observed in traces.
The host pre-transposes x to x^T [d, tokens] so the device never transposes:

  - x^T lives in one flat SBUF buffer [128 d, 16384 t] fp16, loaded by 8 DMAs
    with graded sizes (small first so PE starts early),
  - PE keeps W_eff^T [d, f] stationary and streams x^T 512 tokens at a time
    into PSUM as y^T tiles [f, t] (one flat 8-bank PSUM ring),
  - the scalar (ACT) and vector (DVE) engines alternate PAIRS of groups
    (1024 tokens, 2 PSUM banks per instruction), each doing the fused
    PSUM->SBUF copy + per-partition bias add + fp32->fp16 downconvert,
  - y^T is stored fp16 in 16 pieces of 1024 tokens, each issued as soon as
    its copy-pair lands; the host transposes/upcasts at the end.

HAM-awareness: the PE clock defaults to 1.2 GHz and only reaches 2.4 GHz
after ~3.4 us of SUSTAINED busy (free-running 4096-cycle activity window).
Eight dense warmup matmuls bridge from engine start until the first loads
land, so the un-throttle is granted right as real work begins and the PE
never sits idle >3.4 us until it finishes.

DMA scheduling: the 16 DMA engines fair-share packets across ALL pending
DMAs (measured: not FIFO, even within one queue), and each DMA individually
tops out at ~85-110 GB/s, so saturating the ~427 GB/s aggregate needs 4-6
DMAs in flight at all times.  All 8 x-loads are therefore issued up front
with geometrically graded sizes [1,1,2,3,4,5,7,9] groups: under fair
sharing the small ones complete early (first data ~2 us after the first
packet) and the completion times form an even staircase, giving the PE a
steady supply while the pool stays deep enough to saturate.  Stores are
issued by the sync engine as their copy-pairs land, joining the pool from
the middle of the load phase on.  The scalar engine only loads bias,
pre-triggers the 1.3 us activation-table load with a dummy ACTIVATE, then
runs a pure copy loop.  Each load DMA has its own semaphore; store
completions share one cumulative semaphore waited only at the very end.
"""

import contextlib

import numpy as np

import concourse.bass as bass
import concourse.mybir as mybir
from concourse.bass_utils import run_bass_kernel_spmd

B, S, D = 16, 8192, 128
N_CORES = 8
T = B * S // N_CORES            # tokens per core = 16384
GT = 512                        # tokens per matmul group (= one PSUM bank of fp32)
K_TOT = T // GT                 # 32 groups
N_PS = 8                        # PSUM banks in the ring
N_WARMUP = 8                    # HAM warmup matmuls (~4 us cold, trips un-throttle)

# x-load sizes in groups, graded so fair-shared completions stagger evenly
LOAD_GROUPS = [1, 1, 2, 3, 4, 5, 7, 9]
LOADS = []
_g0 = 0
for _n in LOAD_GROUPS:
    LOADS.append((_g0 * GT, (_g0 + _n) * GT))
    _g0 += _n
LOAD_OF_GROUP = {}
for _i, (_lo, _hi) in enumerate(LOADS):
    for _k in range(_lo // GT, _hi // GT):
        LOAD_OF_GROUP[_k] = _i
# alternate loads across both HWDGE queues: pool reaches full depth ~2x
# sooner than single-queue issue while small loads still deliver first
SYNC_LOAD_ORDER = [0, 2, 4, 6]
ACT_LOAD_ORDER = [1, 3, 5, 7]
# PE dummy-matmul padding before these load-boundary waits: keeps the PE
# dense through mid-run load gaps so HAM never re-throttles it to 1.2 GHz
PAD_AT_GROUP = {}
# store pieces: 16 x 1024 tokens (one copy-pair each).  Small pieces keep the
# pending-DMA pool deep through the store-dominated phase and the tail, which
# matters because per-DMA throughput caps at ~100 GB/s (drift-controlled A/B:
# [2]*16 beat a graded [2,2,4,4,4,4,2,2,2,2,2,2] split in 3/3 rounds).
STORE_GROUPS = [2] * 16
# copy-pair j (groups 2j, 2j+1) -> engine: 'a' = scalar/ACT, 'v' = vector/DVE,
# 'g' = gpsimd.  Each engine's semaphore counts its own completed pair-copies.
PAIR_ENGINE = ['a', 'v'] * (K_TOT // 4)
N_PAIRS = K_TOT // 2
PAIRS_OF = {e: [j for j in range(N_PAIRS) if PAIR_ENGINE[j] == e] for e in 'avg'}


def _pair_count(e, j):
    """How many of engine e's pair-copies are done once pairs 0..j are done."""
    return sum(1 for i in range(j + 1) if PAIR_ENGINE[i] == e)


# issue lag in copy-pairs: store piece p is held until pair p+LAG is copied,
# keeping few stores pending mid-run (loads get the fair-share bandwidth and
# finish sooner) while the clamp lets the tail pieces flood in for pool depth
STORE_LAG = 0
# first-use warm-up knobs: dummy DVE op at stream start; position of ACT's
# table-warm dummy within its load-issue sequence (big number = after all)
WARM_DVE = True
ACT_WARM_AFTER = 1
# fire-and-forget warm-up DMAs, first on each queue: redundantly pre-load the
# first two groups (the gated loads rewrite the same bytes, so no hazard) to
# prime the DMA engines' per-DMA ramp-up during the issue window
WARM_DMA = False


def _build_stores():
    stores = []
    g0 = 0
    for n in STORE_GROUPS:
        g1 = g0 + n
        jmax = g1 // 2 - 1      # last copy-pair this piece needs
        jgate = min(jmax + STORE_LAG, N_PAIRS - 1)
        stores.append((g0, g1, {e: _pair_count(e, jgate) for e in 'avg'}))
        g0 = g1
    assert g0 == K_TOT
    return stores


STORES = _build_stores()
N_ST = len(STORES)
# how many of the LAST store pieces the scalar engine issues after its copy
# loop.  0: scalar-issued tail stores measured SLOWER than sync-issued ones
# (3/3 rounds, +0.7-1.4 us) despite the sync sequencer's ~0.74 us per-issue
# latency gating the tail on paper — keep every store on the sync queue.
N_ST_ACT = 0

F16 = mybir.dt.float16
F32 = mybir.dt.float32


def _build_bass():
    nc = bass.Bass(enable_partition_id=False)
    # x = x^T [d, t]; wt = W_eff^T [d, f]; bias = b_eff [f, 1]; y = y^T [f, t]
    x = nc.dram_tensor("x", [128, T], F16, kind="ExternalInput")
    wt = nc.dram_tensor("wt", [128, 128], F16, kind="ExternalInput")
    bias = nc.dram_tensor("bias", [128, 1], F32, kind="ExternalInput")
    y = nc.dram_tensor("y", [128, T], F16, kind="ExternalOutput")

    with contextlib.ExitStack() as ctx:
        sem = lambda name: ctx.enter_context(nc.semaphore(name))
        sb = lambda name, shape, dt: ctx.enter_context(nc.sbuf_tensor(name, shape, dt))

        s_wb = sem("s_wb")                                # wt + bias loads
        s_l = [sem(f"s_l{i}") for i in range(len(LOADS))] # x loads
        s_mm = sem("s_mm")                                # PE pairs done
        s_cp = {e: sem(f"s_cp{e}") for e in 'avg' if PAIRS_OF[e]}  # pair-copies
        s_out = sem("s_out")                              # store completions
        s_warm = sem("s_warm") if WARM_DMA else None      # warm-DMA sink

        wt_sb = sb("wt_sb", [128, 128], F16)
        bias_sb = sb("bias_sb", [128, 1], F32)
        xbuf = sb("xbuf", [128, T], F16)
        ybuf = sb("ybuf", [128, T], F16)
        psy = ctx.enter_context(nc.psum_tensor("psy", [128, N_PS * GT], F32))

        def bank(i, n=1):
            return psy[:, i * GT:(i + n) * GT]

        with nc.Block() as block:

            @block.sync
            def _(sp: bass.BassEngine):
                if WARM_DMA:
                    sp.dma_start(out=xbuf[:, 0:GT], in_=x[:, 0:GT]).then_inc(s_warm, 16)
                sp.dma_start(out=wt_sb[:, :], in_=wt[:, :]).then_inc(s_wb, 16)
                for i in SYNC_LOAD_ORDER:
                    lo, hi = LOADS[i]
                    sp.dma_start(out=xbuf[:, lo:hi], in_=x[:, lo:hi]).then_inc(s_l[i], 16)
                for g0, g1, need in STORES[:N_ST - N_ST_ACT]:
                    for e in 'avg':
                        if need[e]:
                            sp.wait_ge(s_cp[e], need[e])
                    lo, hi = g0 * GT, g1 * GT
                    sp.dma_start(out=y[:, lo:hi], in_=ybuf[:, lo:hi]).then_inc(s_out, 16)
                sp.wait_ge(s_out, 16 * N_ST)

            @block.tensor
            def _(pe: bass.BassTensorEngine):
                # dense HAM warmup (see module docstring); reads a region only
                # the last load writes (lands several us later) and clobbers
                # bank 7, which real group 7 rewrites later in PE program order.
                for _ in range(N_WARMUP):
                    pe.matmul(out=bank(N_PS - 1), lhsT=wt_sb[:, :],
                              rhs=xbuf[:, T - GT:T], start=True, stop=True)
                pe.wait_ge(s_wb, 32)
                for k in range(K_TOT):
                    if k >= N_PS and k % 2 == 0:
                        # banks k%8, k%8+1 are freed by copy-pair (k-8)//2
                        j = (k - N_PS) // 2
                        e = PAIR_ENGINE[j]
                        pe.wait_ge(s_cp[e], _pair_count(e, j))
                    if k == 0 or LOAD_OF_GROUP[k] != LOAD_OF_GROUP[k - 1]:
                        # dummy matmuls into bank k%8 (already psum-waited, so
                        # free; overwritten by the real group k below) keep the
                        # PE busy while the load lands
                        for _ in range(PAD_AT_GROUP.get(k, 0)):
                            pe.matmul(out=bank(k % N_PS), lhsT=wt_sb[:, :],
                                      rhs=xbuf[:, T - GT:T], start=True, stop=True)
                        pe.wait_ge(s_l[LOAD_OF_GROUP[k]], 16)
                    mm = pe.matmul(
                        out=bank(k % N_PS), lhsT=wt_sb[:, :],
                        rhs=xbuf[:, k * GT:(k + 1) * GT],
                        start=True, stop=True,
                    )
                    if k % 2 == 1:
                        mm.then_inc(s_mm)  # s_mm counts completed PAIRS

            @block.vector
            def _(dve: bass.BassEngine):
                if WARM_DVE:
                    # pre-trigger DVE's first-use ucode/table fetch (first
                    # TENSOR_SCALAR otherwise starts ~3 us after its deps
                    # clear); garbage in/out, overwritten by pair 0's copy
                    dve.tensor_scalar_add(out=ybuf[:, 1:2], in0=ybuf[:, 1:2],
                                          scalar1=bias_sb[:, 0:1])
                # fused 2-bank PSUM->SBUF copy + bias + fp16 cast per pair
                dve.wait_ge(s_wb, 32)
                for j in PAIRS_OF['v']:
                    dve.wait_ge(s_mm, j + 1)
                    dve.tensor_scalar_add(
                        out=ybuf[:, 2 * j * GT:(2 * j + 2) * GT],
                        in0=bank((2 * j) % N_PS, 2),
                        scalar1=bias_sb[:, 0:1],
                    ).then_inc(s_cp['v'])

            if PAIRS_OF['g']:
                @block.gpsimd
                def _(gp: bass.BassGpSimd):
                    gp.wait_ge(s_wb, 32)
                    for j in PAIRS_OF['g']:
                        gp.wait_ge(s_mm, j + 1)
                        gp.tensor_scalar_add(
                            out=ybuf[:, 2 * j * GT:(2 * j + 2) * GT],
                            in0=bank((2 * j) % N_PS, 2),
                            scalar1=bias_sb[:, 0:1],
                        ).then_inc(s_cp['g'])

            @block.scalar
            def _(act: bass.BassScalarEngine):
                if WARM_DMA:
                    act.dma_start(out=xbuf[:, GT:2 * GT], in_=x[:, GT:2 * GT]).then_inc(s_warm, 16)
                act.dma_start(out=bias_sb[:, :], in_=bias[:, :]).then_inc(s_wb, 16)
                for n_issued, i in enumerate(ACT_LOAD_ORDER):
                    if n_issued == ACT_WARM_AFTER:
                        # pre-trigger the 1.3 us activation-table load early
                        # enough that it's done before pair 0's copy is ready
                        act.activation(out=ybuf[:, 0:1], in_=ybuf[:, 0:1],
                                       func=mybir.ActivationFunctionType.Identity,
                                       bias=0.0)
                    lo, hi = LOADS[i]
                    act.dma_start(out=xbuf[:, lo:hi], in_=x[:, lo:hi]).then_inc(s_l[i], 16)
                if ACT_WARM_AFTER >= len(ACT_LOAD_ORDER):
                    act.activation(out=ybuf[:, 0:1], in_=ybuf[:, 0:1],
                                   func=mybir.ActivationFunctionType.Identity,
                                   bias=0.0)
                act.wait_ge(s_wb, 32)
                # fused 2-bank copy via activation per pair
                for j in PAIRS_OF['a']:
                    act.wait_ge(s_mm, j + 1)
                    act.activation(
                        out=ybuf[:, 2 * j * GT:(2 * j + 2) * GT],
                        in_=bank((2 * j) % N_PS, 2),
                        func=mybir.ActivationFunctionType.Identity,
                        bias=bias_sb[:, 0:1],
                    ).then_inc(s_cp['a'])
                # tail store pieces, issued here after the copy loop so the
                # sync sequencer's per-issue latency stops gating the tail
                for g0, g1, need in STORES[N_ST - N_ST_ACT:]:
                    for e in 'avg':
                        if need[e]:
                            act.wait_ge(s_cp[e], need[e])
                    lo, hi = g0 * GT, g1 * GT
                    act.dma_start(out=y[:, lo:hi], in_=ybuf[:, lo:hi]).then_inc(s_out, 16)

    return nc


_NC_CACHE = {}


def _get_nc():
    if "nc" not in _NC_CACHE:
        _NC_CACHE["nc"] = _build_bass()
    return _NC_CACHE["nc"]


def _prep_consts(W, b, weights):
    W64 = np.asarray(W, dtype=np.float64)
    b64 = np.asarray(b, dtype=np.float64)
    w64 = np.asarray(weights, dtype=np.float64)
    w_eff = np.einsum("o,ofd->fd", w64, W64)                       # [f, d]
    b_eff = w64 @ b64                                              # [f]
    wt16 = np.ascontiguousarray(w_eff.T.astype(np.float16))        # [d, f]
    bias32 = np.ascontiguousarray(b_eff.astype(np.float32).reshape(D, 1))
    return wt16, bias32


def _make_in_maps(x, W, b, weights):
    x = np.asarray(x, dtype=np.float32).reshape(B * S, D)
    wt16, bias32 = _prep_consts(W, b, weights)
    xT = x.T.astype(np.float16)                                    # [d, tokens]
    shards = np.ascontiguousarray(xT.reshape(D, N_CORES, T).transpose(1, 0, 2))
    return [{"x": shards[i], "wt": wt16, "bias": bias32} for i in range(N_CORES)]


def _assemble(results):
    yT = np.stack([results[i]["y"] for i in range(N_CORES)])       # [core, f, t] fp16
    y = yT.transpose(0, 2, 1).reshape(B * S, D).astype(np.float32)
    return y.reshape(B, S, D)


def kernel(x, W, b, weights):
    nc = _get_nc()
    res = run_bass_kernel_spmd(nc, _make_in_maps(x, W, b, weights),
                               list(range(N_CORES)))
    return _assemble(res.results)


def kernel_profiled(x, W, b, weights, **kw):
    """Same as kernel() but traces; returns (y, BassKernelResults)."""
    nc = _get_nc()
    res = run_bass_kernel_spmd(nc, _make_in_maps(x, W, b, weights),
                               list(range(N_CORES)), trace=True, **kw)
    return _assemble(res.results), res


# revision 45
# speedup vs baseline: 1.0929x; 1.0929x over previous
"""Trainium2 Bass kernel for nn_MixModule (moe_routing).

Math: the reference computes outs[b,s,o,f] = sum_d x[b,s,d]*W[o,f,d] + b[o,f],
then y = sum_o weights[o]*outs[...,o,:].  This collapses to a single affine map

    W_eff[f,d] = sum_o weights[o] * W[o,f,d]
    b_eff[f]   = sum_o weights[o] * b[o,f]
    y          = x @ W_eff.T + b_eff

Sharding: data-parallel over tokens, 16384 tokens per core across 8 cores;
W/b replicated; no cross-core communication.

fp16 end-to-end on the wire (rel-err budget is 2e-2; fp16 quantization of x
and y costs ~4e-4), which halves HBM traffic vs fp32: 4 MiB in + 4 MiB out
per core ~= 19.7 us at the ~427 GB/s/core DMA ceiling observed in traces.
The host pre-transposes x to x^T [d, tokens] so the device never transposes:

  - x^T lives in one flat SBUF buffer [128 d, 16384 t] fp16, loaded by 8 DMAs
    with graded sizes (small first so PE starts early),
  - PE keeps W_eff^T [d, f] stationary and streams x^T 512 tokens at a time
    into PSUM as y^T tiles [f, t] (one flat 8-bank PSUM ring),
  - the scalar (ACT) and vector (DVE) engines alternate PAIRS of groups
    (1024 tokens, 2 PSUM banks per instruction), each doing the fused
    PSUM->SBUF copy + per-partition bias add + fp32->fp16 downconvert,
  - y^T is stored fp16 in 16 pieces of 1024 tokens, each issued as soon as
    its copy-pair lands; the host transposes/upcasts at the end.

HAM-awareness: the PE clock defaults to 1.2 GHz and only reaches 2.4 GHz
after ~3.4 us of SUSTAINED busy (free-running 4096-cycle activity window).
Eight dense warmup matmuls bridge from engine start until the first loads
land, so the un-throttle is granted right as real work begins and the PE
never sits idle >3.4 us until it finishes.

DMA scheduling: the 16 DMA engines fair-share packets across ALL pending
DMAs (measured: not FIFO, even within one queue), and each DMA individually
tops out at ~85-110 GB/s, so saturating the ~427 GB/s aggregate needs 4-6
DMAs in flight at all times.  All 8 x-loads are therefore issued up front
with geometrically graded sizes [1,1,2,3,4,5,7,9] groups: under fair
sharing the small ones complete early (first data ~2 us after the first
packet) and the completion times form an even staircase, giving the PE a
steady supply while the pool stays deep enough to saturate.  Stores are
issued by the sync engine as their copy-pairs land, joining the pool from
the middle of the load phase on.  The scalar engine only loads bias,
pre-triggers the 1.3 us activation-table load with a dummy ACTIVATE, then
runs a pure copy loop.  Each load DMA has its own semaphore; store
completions share one cumulative semaphore waited only at the very end.
"""

import contextlib

import numpy as np

import concourse.bass as bass
import concourse.mybir as mybir
from concourse.bass_utils import run_bass_kernel_spmd

B, S, D = 16, 8192, 128
N_CORES = 8
T = B * S // N_CORES            # tokens per core = 16384
GT = 512                        # tokens per matmul group (= one PSUM bank of fp32)
K_TOT = T // GT                 # 32 groups
N_PS = 8                        # PSUM banks in the ring
N_WARMUP = 8                    # HAM warmup matmuls (~4 us cold, trips un-throttle)

# x-load sizes in groups, graded so fair-shared completions stagger evenly
LOAD_GROUPS = [1, 1, 2, 3, 4, 5, 7, 9]
LOADS = []
_g0 = 0
for _n in LOAD_GROUPS:
    LOADS.append((_g0 * GT, (_g0 + _n) * GT))
    _g0 += _n
LOAD_OF_GROUP = {}
for _i, (_lo, _hi) in enumerate(LOADS):
    for _k in range(_lo // GT, _hi // GT):
        LOAD_OF_GROUP[_k] = _i
# alternate loads across both HWDGE queues: pool reaches full depth ~2x
# sooner than single-queue issue while small loads still deliver first
SYNC_LOAD_ORDER = [0, 2, 4, 6]
ACT_LOAD_ORDER = [1, 3, 5, 7]
# PE dummy-matmul padding before these load-boundary waits: keeps the PE
# dense through mid-run load gaps so HAM never re-throttles it to 1.2 GHz
PAD_AT_GROUP = {}
# store pieces: 16 x 1024 tokens (one copy-pair each).  Small pieces keep the
# pending-DMA pool deep through the store-dominated phase and the tail, which
# matters because per-DMA throughput caps at ~100 GB/s (drift-controlled A/B:
# [2]*16 beat a graded [2,2,4,4,4,4,2,2,2,2,2,2] split in 3/3 rounds).
STORE_GROUPS = [2] * 16
# copy-pair j (groups 2j, 2j+1) -> engine: 'a' = scalar/ACT, 'v' = vector/DVE,
# 'g' = gpsimd.  Each engine's semaphore counts its own completed pair-copies.
# tail swapped so the slower DVE (1.27 us/pair vs ACT 1.11) does not hold the
# final pair: last copies finish ~1 us sooner (A/B win with NO_FINAL_WAIT)
PAIR_ENGINE = ['a', 'v'] * (K_TOT // 4 - 2) + ['v', 'a', 'v', 'a']
N_PAIRS = K_TOT // 2
PAIRS_OF = {e: [j for j in range(N_PAIRS) if PAIR_ENGINE[j] == e] for e in 'avg'}


def _pair_count(e, j):
    """How many of engine e's pair-copies are done once pairs 0..j are done."""
    return sum(1 for i in range(j + 1) if PAIR_ENGINE[i] == e)


# issue lag in copy-pairs: store piece p is held until pair p+LAG is copied,
# keeping few stores pending mid-run (loads get the fair-share bandwidth and
# finish sooner) while the clamp lets the tail pieces flood in for pool depth
STORE_LAG = 0
# first-use warm-up knobs: dummy DVE op at stream start; position of ACT's
# table-warm dummy within its load-issue sequence (big number = after all)
WARM_DVE = True
ACT_WARM_AFTER = 1
# fire-and-forget warm-up DMAs, first on each queue: redundantly pre-load the
# first two groups (the gated loads rewrite the same bytes, so no hazard) to
# prime the DMA engines' per-DMA ramp-up during the issue window
WARM_DMA = False
# skip the sync engine's final s_out wait.  MUST STAY False: without the
# wait a fresh-process run returned rel err 2.2e-2 (stale store data) even
# though three in-process A/B runs passed — engine-retire dge_drain does NOT
# reliably cover store completion before the host reads the output
NO_FINAL_WAIT = False


def _build_stores():
    stores = []
    g0 = 0
    for n in STORE_GROUPS:
        g1 = g0 + n
        jmax = g1 // 2 - 1      # last copy-pair this piece needs
        jgate = min(jmax + STORE_LAG, N_PAIRS - 1)
        stores.append((g0, g1, {e: _pair_count(e, jgate) for e in 'avg'}))
        g0 = g1
    assert g0 == K_TOT
    return stores


STORES = _build_stores()
N_ST = len(STORES)
# how many of the LAST store pieces the scalar engine issues after its copy
# loop.  0: scalar-issued tail stores measured SLOWER than sync-issued ones
# (3/3 rounds, +0.7-1.4 us) despite the sync sequencer's ~0.74 us per-issue
# latency gating the tail on paper — keep every store on the sync queue.
N_ST_ACT = 0

F16 = mybir.dt.float16
F32 = mybir.dt.float32


def _build_bass():
    nc = bass.Bass(enable_partition_id=False)
    # x = x^T [d, t]; wt = W_eff^T [d, f]; bias = b_eff [f, 1]; y = y^T [f, t]
    x = nc.dram_tensor("x", [128, T], F16, kind="ExternalInput")
    wt = nc.dram_tensor("wt", [128, 128], F16, kind="ExternalInput")
    bias = nc.dram_tensor("bias", [128, 1], F32, kind="ExternalInput")
    y = nc.dram_tensor("y", [128, T], F16, kind="ExternalOutput")

    with contextlib.ExitStack() as ctx:
        sem = lambda name: ctx.enter_context(nc.semaphore(name))
        sb = lambda name, shape, dt: ctx.enter_context(nc.sbuf_tensor(name, shape, dt))

        s_wb = sem("s_wb")                                # wt + bias loads
        s_l = [sem(f"s_l{i}") for i in range(len(LOADS))] # x loads
        s_mm = sem("s_mm")                                # PE pairs done
        s_cp = {e: sem(f"s_cp{e}") for e in 'avg' if PAIRS_OF[e]}  # pair-copies
        s_out = sem("s_out")                              # store completions
        s_warm = sem("s_warm") if WARM_DMA else None      # warm-DMA sink

        wt_sb = sb("wt_sb", [128, 128], F16)
        bias_sb = sb("bias_sb", [128, 1], F32)
        xbuf = sb("xbuf", [128, T], F16)
        ybuf = sb("ybuf", [128, T], F16)
        psy = ctx.enter_context(nc.psum_tensor("psy", [128, N_PS * GT], F32))

        def bank(i, n=1):
            return psy[:, i * GT:(i + n) * GT]

        with nc.Block() as block:

            @block.sync
            def _(sp: bass.BassEngine):
                if WARM_DMA:
                    sp.dma_start(out=xbuf[:, 0:GT], in_=x[:, 0:GT]).then_inc(s_warm, 16)
                sp.dma_start(out=wt_sb[:, :], in_=wt[:, :]).then_inc(s_wb, 16)
                for i in SYNC_LOAD_ORDER:
                    lo, hi = LOADS[i]
                    sp.dma_start(out=xbuf[:, lo:hi], in_=x[:, lo:hi]).then_inc(s_l[i], 16)
                for g0, g1, need in STORES[:N_ST - N_ST_ACT]:
                    for e in 'avg':
                        if need[e]:
                            sp.wait_ge(s_cp[e], need[e])
                    lo, hi = g0 * GT, g1 * GT
                    sp.dma_start(out=y[:, lo:hi], in_=ybuf[:, lo:hi]).then_inc(s_out, 16)
                if not NO_FINAL_WAIT:
                    sp.wait_ge(s_out, 16 * N_ST)

            @block.tensor
            def _(pe: bass.BassTensorEngine):
                # dense HAM warmup (see module docstring); reads a region only
                # the last load writes (lands several us later) and clobbers
                # bank 7, which real group 7 rewrites later in PE program order.
                for _ in range(N_WARMUP):
                    pe.matmul(out=bank(N_PS - 1), lhsT=wt_sb[:, :],
                              rhs=xbuf[:, T - GT:T], start=True, stop=True)
                pe.wait_ge(s_wb, 32)
                for k in range(K_TOT):
                    if k >= N_PS and k % 2 == 0:
                        # banks k%8, k%8+1 are freed by copy-pair (k-8)//2
                        j = (k - N_PS) // 2
                        e = PAIR_ENGINE[j]
                        pe.wait_ge(s_cp[e], _pair_count(e, j))
                    if k == 0 or LOAD_OF_GROUP[k] != LOAD_OF_GROUP[k - 1]:
                        # dummy matmuls into bank k%8 (already psum-waited, so
                        # free; overwritten by the real group k below) keep the
                        # PE busy while the load lands
                        for _ in range(PAD_AT_GROUP.get(k, 0)):
                            pe.matmul(out=bank(k % N_PS), lhsT=wt_sb[:, :],
                                      rhs=xbuf[:, T - GT:T], start=True, stop=True)
                        pe.wait_ge(s_l[LOAD_OF_GROUP[k]], 16)
                    mm = pe.matmul(
                        out=bank(k % N_PS), lhsT=wt_sb[:, :],
                        rhs=xbuf[:, k * GT:(k + 1) * GT],
                        start=True, stop=True,
                    )
                    if k % 2 == 1:
                        mm.then_inc(s_mm)  # s_mm counts completed PAIRS

            @block.vector
            def _(dve: bass.BassEngine):
                if WARM_DVE:
                    # pre-trigger DVE's first-use ucode/table fetch (first
                    # TENSOR_SCALAR otherwise starts ~3 us after its deps
                    # clear); garbage in/out, overwritten by pair 0's copy
                    dve.tensor_scalar_add(out=ybuf[:, 1:2], in0=ybuf[:, 1:2],
                                          scalar1=bias_sb[:, 0:1])
                # fused 2-bank PSUM->SBUF copy + bias + fp16 cast per pair
                dve.wait_ge(s_wb, 32)
                for j in PAIRS_OF['v']:
                    dve.wait_ge(s_mm, j + 1)
                    dve.tensor_scalar_add(
                        out=ybuf[:, 2 * j * GT:(2 * j + 2) * GT],
                        in0=bank((2 * j) % N_PS, 2),
                        scalar1=bias_sb[:, 0:1],
                    ).then_inc(s_cp['v'])

            if PAIRS_OF['g']:
                @block.gpsimd
                def _(gp: bass.BassGpSimd):
                    gp.wait_ge(s_wb, 32)
                    for j in PAIRS_OF['g']:
                        gp.wait_ge(s_mm, j + 1)
                        gp.tensor_scalar_add(
                            out=ybuf[:, 2 * j * GT:(2 * j + 2) * GT],
                            in0=bank((2 * j) % N_PS, 2),
                            scalar1=bias_sb[:, 0:1],
                        ).then_inc(s_cp['g'])

            @block.scalar
            def _(act: bass.BassScalarEngine):
                if WARM_DMA:
                    act.dma_start(out=xbuf[:, GT:2 * GT], in_=x[:, GT:2 * GT]).then_inc(s_warm, 16)
                act.dma_start(out=bias_sb[:, :], in_=bias[:, :]).then_inc(s_wb, 16)
                for n_issued, i in enumerate(ACT_LOAD_ORDER):
                    if n_issued == ACT_WARM_AFTER:
                        # pre-trigger the 1.3 us activation-table load early
                        # enough that it's done before pair 0's copy is ready
                        act.activation(out=ybuf[:, 0:1], in_=ybuf[:, 0:1],
                                       func=mybir.ActivationFunctionType.Identity,
                                       bias=0.0)
                    lo, hi = LOADS[i]
                    act.dma_start(out=xbuf[:, lo:hi], in_=x[:, lo:hi]).then_inc(s_l[i], 16)
                if ACT_WARM_AFTER >= len(ACT_LOAD_ORDER):
                    act.activation(out=ybuf[:, 0:1], in_=ybuf[:, 0:1],
                                   func=mybir.ActivationFunctionType.Identity,
                                   bias=0.0)
                act.wait_ge(s_wb, 32)
                # fused 2-bank copy via activation per pair
                for j in PAIRS_OF['a']:
                    act.wait_ge(s_mm, j + 1)
                    act.activation(
                        out=ybuf[:, 2 * j * GT:(2 * j + 2) * GT],
                        in_=bank((2 * j) % N_PS, 2),
                        func=mybir.ActivationFunctionType.Identity,
                        bias=bias_sb[:, 0:1],
                    ).then_inc(s_cp['a'])
                # tail store pieces, issued here after the copy loop so the
                # sync sequencer's per-issue latency stops gating the tail
                for g0, g1, need in STORES[N_ST - N_ST_ACT:]:
                    for e in 'avg':
                        if need[e]:
                            act.wait_ge(s_cp[e], need[e])
                    lo, hi = g0 * GT, g1 * GT
                    act.dma_start(out=y[:, lo:hi], in_=ybuf[:, lo:hi]).then_inc(s_out, 16)

    return nc


_NC_CACHE = {}


def _get_nc():
    if "nc" not in _NC_CACHE:
        _NC_CACHE["nc"] = _build_bass()
    return _NC_CACHE["nc"]


def _prep_consts(W, b, weights):
    W64 = np.asarray(W, dtype=np.float64)
    b64 = np.asarray(b, dtype=np.float64)
    w64 = np.asarray(weights, dtype=np.float64)
    w_eff = np.einsum("o,ofd->fd", w64, W64)                       # [f, d]
    b_eff = w64 @ b64                                              # [f]
    wt16 = np.ascontiguousarray(w_eff.T.astype(np.float16))        # [d, f]
    bias32 = np.ascontiguousarray(b_eff.astype(np.float32).reshape(D, 1))
    return wt16, bias32


def _make_in_maps(x, W, b, weights):
    x = np.asarray(x, dtype=np.float32).reshape(B * S, D)
    wt16, bias32 = _prep_consts(W, b, weights)
    xT = x.T.astype(np.float16)                                    # [d, tokens]
    shards = np.ascontiguousarray(xT.reshape(D, N_CORES, T).transpose(1, 0, 2))
    return [{"x": shards[i], "wt": wt16, "bias": bias32} for i in range(N_CORES)]


def _assemble(results):
    yT = np.stack([results[i]["y"] for i in range(N_CORES)])       # [core, f, t] fp16
    y = yT.transpose(0, 2, 1).reshape(B * S, D).astype(np.float32)
    return y.reshape(B, S, D)


def kernel(x, W, b, weights):
    nc = _get_nc()
    res = run_bass_kernel_spmd(nc, _make_in_maps(x, W, b, weights),
                               list(range(N_CORES)))
    return _assemble(res.results)


def kernel_profiled(x, W, b, weights, **kw):
    """Same as kernel() but traces; returns (y, BassKernelResults)."""
    nc = _get_nc()
    res = run_bass_kernel_spmd(nc, _make_in_maps(x, W, b, weights),
                               list(range(N_CORES)), trace=True, **kw)
    return _assemble(res.results), res
